# revision 1
# baseline (speedup 1.0000x reference)
"""Dcls2_1d (dilated conv with learnable row spacings) on 8 trn2 NeuronCores.

Strategy: data-parallel over batch (16 -> 2 images/core). Host constructs the
dense (O, I, 7, 3) scattered kernel (exact port of the reference bilinear
scatter, ~0.7 MFLOP) and pads x; each core runs the conv as an implicit GEMM
contracting over C_in=128 (the partition dim), with the bias fused into the
PSUM evacuation.

Default path (DCLS_ALGO=wino, DCLS_DT=fp16): Winograd F(2,3) over the width
taps — 4 multiply-points per 2 output columns instead of 6, cutting the PE's
streamed matmul columns by 1/3. The input transform (+-1 butterflies) and the
output assembly run on the otherwise-idle DVE; PSUM is only ever read by the
scalar engine (PE-write + DVE-read on one PSUM bank is fatal on TRN2 HW).
The 7 height taps stay direct, accumulated in PSUM per 15-row strip.

Fallbacks via env: DCLS_ALGO=direct (21-tap dense GEMM), DCLS_DT=f32r
(~tf32-precision matmuls, rel err 1.4e-4 vs fp16's 4.2e-4, ~1.5x slower).

Input DMAs are priority-ordered (first strip's operands first, interleaved so
the matmul stream never catches up), issued from three engine queues in
parallel, and spread over the 16 HW DMA queues (~22 GB/s each). A short burst
of dummy matmuls warms the PE clock gate (HAM) while inputs are in flight.

Measured on trn2: ~118 us/core HW exec (PE streaming floor ~96 us), max-abs
rel err 4.2e-4 vs the fp32 reference.
"""
import os
import sys
import time

sys.path.insert(0, "/opt/trn_rl_repo")

import ml_dtypes
import numpy as np

import concourse.bass as bass
import concourse.tile as tile
from concourse import bacc, mybir
from concourse import bass_utils

# ---- problem constants (hardcoded per contract) ----
K_H, K_W = 3, 3
LIM = 2            # DIL // 2
KH_EFF = 7         # K_H + 2 * LIM
PAD_H, PAD_W = 3, 1
B, CIN, H, W = 16, 128, 64, 64
COUT = 256
N_CORES = 8
BPC = B // N_CORES                  # images per core
HP, WP = H + 2 * PAD_H, W + 2 * PAD_W   # 70, 66
NPIX = H * W                        # 4096
CHUNK = 512                         # output pixels per PSUM bank
NCHUNK = NPIX // CHUNK              # 8
RPC = CHUNK // W                    # rows per chunk: 8
NTAPS = KH_EFF * K_W                # 21
OH = COUT // 128                    # 2 halves of out channels

DT = os.environ.get("DCLS_DT", "fp16")          # f32r | fp16 | bf16 | f32
ALGO = os.environ.get("DCLS_ALGO", "wino")       # wino | direct
ORDER = os.environ.get("DCLS_ORDER", "chunk")    # chunk | tap
WARMUP = int(os.environ.get("DCLS_WARMUP", "10"))
_MM_DT = {"f32r": mybir.dt.float32r, "fp16": mybir.dt.float16,
          "bf16": mybir.dt.bfloat16, "f32": mybir.dt.float32}[DT]
_NP_DT = {"f32r": np.float32, "fp16": np.float16,
          "bf16": ml_dtypes.bfloat16, "f32": np.float32}[DT]

_NC_CACHE = None
_last_in_maps = None  # stashed for test.py's profiled re-run


def _build_kernel_np(weight: np.ndarray, P1: np.ndarray) -> np.ndarray:
    """Exact numpy port of reference.build_kernel (fp32)."""
    weight = weight.astype(np.float32, copy=False)
    kh = np.arange(K_H, dtype=np.float32)[None, None, :, None]
    pos = kh + LIM + np.clip(P1.astype(np.float32, copy=False), -LIM, LIM)
    p0 = np.floor(pos)
    frac = pos - p0
    p0i = p0.astype(np.int32)
    rng = np.arange(KH_EFF, dtype=np.int32)
    oh0 = (p0i[..., None] == rng).astype(np.float32)
    oh1 = ((p0i + 1)[..., None] == rng).astype(np.float32)
    return (
        np.einsum("oihw,oihwk->oikw", weight * (1.0 - frac), oh0)
        + np.einsum("oihw,oihwk->oikw", weight * frac, oh1)
    ).astype(np.float32)


def _splits(total, n):
    """n near-equal [lo, hi) column ranges covering [0, total)."""
    step = (total + n - 1) // n
    return [(j, min(j + step, total)) for j in range(0, total, step)]


def _build_bass():
    mmdt = _MM_DT
    f32 = mybir.dt.float32
    nc = bacc.Bacc("TRN2", target_bir_lowering=False, debug=False,
                   num_devices=N_CORES)
    x_d = nc.dram_tensor("x", [BPC, CIN, HP * WP], mmdt,
                         kind="ExternalInput").ap()
    # oh-major weight layout: [i, (oh, kh, kw, o128)]
    k_d = nc.dram_tensor("k", [CIN, OH * NTAPS * 128], mmdt,
                         kind="ExternalInput").ap()
    b_d = nc.dram_tensor("b", [OH, 128, 1], f32, kind="ExternalInput").ap()
    o_d = nc.dram_tensor("o", [BPC, OH, 128, NPIX], f32,
                         kind="ExternalOutput").ap()

    HEAD_ROWS = RPC + KH_EFF - 1            # x rows needed by first chunk: 14
    HEAD = HEAD_ROWS * WP                   # 924 cols

    # DMA descriptor issue costs ~0.6us on an engine queue; spread issues
    # over four otherwise-idle engine queues so they go out in parallel.
    _rr = [0]

    def dma(engines, dst, src):
        eng = engines[_rr[0] % len(engines)]
        _rr[0] += 1
        eng.dma_start(dst, src)

    with tile.TileContext(nc) as tc:
        with tc.tile_pool(name="xp", bufs=1) as xp, \
             tc.tile_pool(name="kp", bufs=1) as kp, \
             tc.tile_pool(name="bp", bufs=1) as bp, \
             tc.tile_pool(name="wu", bufs=1) as wu, \
             tc.tile_pool(name="ps", bufs=8, space="PSUM") as ps, \
             tc.tile_pool(name="op", bufs=4) as op:

            kt = kp.tile([CIN, OH * NTAPS * 128], mmdt, tag="k")
            bt = bp.tile([128, OH], f32, tag="bias")
            xts = [xp.tile([CIN, HP * WP], mmdt, tag=f"x{n}", name=f"x{n}")
                   for n in range(BPC)]

            # warmup tile for the PE clock (HAM) ramp: memset-fed fp32
            # (no DMA deps) so the dummy matmuls run while the real inputs
            # are still in flight; their PSUM output is never read
            wt = None
            if WARMUP:
                wt = wu.tile([128, 128], f32, tag="warm")
                nc.vector.memset(wt[:], 0.0)

            # --- input DMAs, priority-ordered, issued from 4 engines in
            # parallel, spread over the 16 HW queues ---
            ie = [nc.sync, nc.gpsimd, nc.scalar]
            # 1) first rows of image 0 (first matmul needs them + tap0 weights)
            for lo, hi in _splits(HEAD, 8):
                dma(ie, xts[0][:, lo:hi], x_d[0][:, lo:hi])
            # 2) weights for the first oh half, fine-grained so taps stream in
            for lo, hi in _splits(NTAPS * 128, 16):
                dma(ie, kt[:, lo:hi], k_d[:, lo:hi])
            # 3) rest of image 0
            for lo, hi in _splits(HP * WP - HEAD, 5):
                dma(ie, xts[0][:, HEAD + lo:HEAD + hi],
                    x_d[0][:, HEAD + lo:HEAD + hi])
            # 4) bias, second weight half, remaining images
            for h in range(OH):
                dma(ie, bt[:, h:h + 1], b_d[h])
            for lo, hi in _splits(NTAPS * 128, 8):
                off = NTAPS * 128
                dma(ie, kt[:, off + lo:off + hi], k_d[:, off + lo:off + hi])
            for n in range(1, BPC):
                for lo, hi in _splits(HP * WP, 6):
                    dma(ie, xts[n][:, lo:hi], x_d[n][:, lo:hi])

            # --- HAM warmup: dummy matmuls while inputs stream in ---
            for _ in range(WARMUP):
                pw = ps.tile([128, 128], f32, tag="acc")
                nc.tensor.matmul(pw[:], wt[:], wt[:], start=True,
                                 stop=True)

            # --- the conv ---
            def do_group(n, h, c, xv):
                pt = ps.tile([128, CHUNK], f32, tag="acc")
                y0 = c * RPC
                for t, (kh, kw) in enumerate(
                        (kh, kw) for kh in range(KH_EFF)
                        for kw in range(K_W)):
                    rhs = xv[:, y0 + kh:y0 + kh + RPC, kw:kw + W]
                    off = ((h * KH_EFF + kh) * K_W + kw) * 128
                    nc.tensor.matmul(pt[:], kt[:, off:off + 128], rhs,
                                     start=(t == 0), stop=(t == NTAPS - 1))
                ot = op.tile([128, CHUNK], f32, tag="out")
                nc.scalar.activation(ot[:], pt[:],
                                     mybir.ActivationFunctionType.Identity,
                                     bias=bt[:, h:h + 1])
                # split the store so the flush of the last chunk isn't
                # bottlenecked on a single ~22GB/s DMA queue; the very last
                # store goes 8-way on the HW queues (SW queues drain slowly)
                last = (n == BPC - 1 and h == OH - 1 and c == NCHUNK - 1)
                oe = [nc.sync, nc.scalar] if last else [nc.sync, nc.gpsimd]
                for lo, hi in _splits(CHUNK, 8 if last else 2):
                    dma(oe, o_d[n, h][:, c * CHUNK + lo:c * CHUNK + hi],
                        ot[:, lo:hi])

            def do_block_tap_outer(n, h, xv):
                pts = [ps.tile([128, CHUNK], f32, tag="acc",
                               name=f"acc_{n}_{h}_{c}")
                       for c in range(NCHUNK)]
                for t, (kh, kw) in enumerate(
                        (kh, kw) for kh in range(KH_EFF)
                        for kw in range(K_W)):
                    off = ((h * KH_EFF + kh) * K_W + kw) * 128
                    for c in range(NCHUNK):
                        rhs = xv[:, c * RPC + kh:c * RPC + kh + RPC, kw:kw + W]
                        nc.tensor.matmul(pts[c][:], kt[:, off:off + 128], rhs,
                                         start=(t == 0),
                                         stop=(t == NTAPS - 1))
                for c in range(NCHUNK):
                    ot = op.tile([128, CHUNK], f32, tag="out")
                    nc.scalar.activation(ot[:], pts[c][:],
                                         mybir.ActivationFunctionType.Identity,
                                         bias=bt[:, h:h + 1])
                    last = (n == BPC - 1 and h == OH - 1 and c == NCHUNK - 1)
                    oe = [nc.sync, nc.gpsimd]
                    for lo, hi in _splits(CHUNK, 4 if last else 2):
                        dma(oe, o_d[n, h][:, c * CHUNK + lo:c * CHUNK + hi],
                            ot[:, lo:hi])

            for n in range(BPC):
                xv = xts[n][:].rearrange("p (h w) -> p h w", h=HP)
                for h in range(OH):
                    if ORDER == "tap":
                        do_block_tap_outer(n, h, xv)
                    else:
                        for c in range(NCHUNK):
                            do_group(n, h, c, xv)
    t0 = time.time()
    nc.compile()
    print(f"[kernel] bacc compile: {time.time()-t0:.1f}s", file=sys.stderr)
    return nc


NJ = 4                       # Winograd F(2,3) points over kw
PAIRS = W // 2               # output column pairs: 32
STRIPS = [(0, 15), (15, 30), (30, 45), (45, 60), (60, 64)]
RB = [(0, 18), (18, 36), (36, 54), (54, 70)]   # input-transform row blocks


def _build_bass_wino():
    """Winograd F(2,3) over the width taps: out cols (2p, 2p+1) come from
    4 multiply-points j on input cols (2p..2p+3), so the PE streams 4/6 of
    the direct method's columns. Transforms run on the otherwise-idle
    DVE (input, output assembly) and ACT (bias) engines.

      W0 = d0-d2, W1 = d1+d2, W2 = d2-d1, W3 = d1-d3     (input, DVE)
      o_even = m0+m1+m2,  o_odd = m1-m2-m3               (output, DVE)
    """
    mmdt = _MM_DT
    f32 = mybir.dt.float32
    nc = bacc.Bacc("TRN2", target_bir_lowering=False, debug=False,
                   num_devices=N_CORES)
    x_d = nc.dram_tensor("x", [BPC, CIN, HP * WP], mmdt,
                         kind="ExternalInput").ap()
    # transformed weights: [i, (oh, j, kh, o128)]
    KCOLS = OH * NJ * KH_EFF * 128
    k_d = nc.dram_tensor("k", [CIN, KCOLS], mmdt, kind="ExternalInput").ap()
    b_d = nc.dram_tensor("b", [OH, 128, 1], f32, kind="ExternalInput").ap()
    o_d = nc.dram_tensor("o", [BPC, OH, 128, NPIX], f32,
                         kind="ExternalOutput").ap()

    _rr = [0]

    def dma(engines, dst, src):
        eng = engines[_rr[0] % len(engines)]
        _rr[0] += 1
        eng.dma_start(dst, src)

    HEAD = RB[0][1] * WP      # x cols needed by the first transform block

    with tile.TileContext(nc) as tc:
        with tc.tile_pool(name="xp", bufs=1) as xp, \
             tc.tile_pool(name="wp", bufs=1) as wpool, \
             tc.tile_pool(name="kp", bufs=1) as kp, \
             tc.tile_pool(name="bp", bufs=1) as bp, \
             tc.tile_pool(name="wu", bufs=1) as wu, \
             tc.tile_pool(name="ps", bufs=8, space="PSUM") as ps, \
             tc.tile_pool(name="ev", bufs=8) as ev, \
             tc.tile_pool(name="op", bufs=4) as op:

            kt = kp.tile([CIN, KCOLS], mmdt, tag="k")
            bt = bp.tile([128, OH], f32, tag="bias")
            xts = [xp.tile([CIN, HP * WP], mmdt, tag=f"x{n}", name=f"x{n}")
                   for n in range(BPC)]
            wts = [wpool.tile([CIN, NJ * HP * PAIRS], mmdt, tag=f"w{n}",
                              name=f"w{n}")
                   for n in range(BPC)]

            wt = None
            if WARMUP:
                wt = wu.tile([128, 128], f32, tag="warm")
                nc.vector.memset(wt[:], 0.0)

            # --- input DMAs, priority-ordered ---
            ie = [nc.sync, nc.gpsimd, nc.scalar]
            # first two transform blocks of image 0 (strip 1 consumes block
            # 1's rows ~6us after the first matmul), with the first oh half
            # of the weights (fully consumed by strip 0) interleaved so the
            # matmul stream doesn't catch up to either
            ksp = _splits(KCOLS // 2, 12)
            for lo, hi in _splits(HEAD, 6):
                dma(ie, xts[0][:, lo:hi], x_d[0][:, lo:hi])
            for lo, hi in ksp[:5]:
                dma(ie, kt[:, lo:hi], k_d[:, lo:hi])
            B1 = RB[1][1] * WP
            for lo, hi in _splits(B1 - HEAD, 4):
                dma(ie, xts[0][:, HEAD + lo:HEAD + hi],
                    x_d[0][:, HEAD + lo:HEAD + hi])
            for lo, hi in ksp[5:]:
                dma(ie, kt[:, lo:hi], k_d[:, lo:hi])
            # rest of image 0
            for lo, hi in _splits(HP * WP - B1, 5):
                dma(ie, xts[0][:, B1 + lo:B1 + hi],
                    x_d[0][:, B1 + lo:B1 + hi])
            for h in range(OH):
                dma(ie, bt[:, h:h + 1], b_d[h])
            for lo, hi in _splits(KCOLS // 2, 8):
                off = KCOLS // 2
                dma(ie, kt[:, off + lo:off + hi], k_d[:, off + lo:off + hi])
            for n in range(1, BPC):
                for lo, hi in _splits(HP * WP, 6):
                    dma(ie, xts[n][:, lo:hi], x_d[n][:, lo:hi])

            # --- HAM warmup ---
            for _ in range(WARMUP):
                pw = ps.tile([128, 128], f32, tag="acc")
                nc.tensor.matmul(pw[:], wt[:], wt[:], start=True, stop=True)

            xvs = [xts[n][:].rearrange("p (r c) -> p r c", r=HP)
                   for n in range(BPC)]
            wvs = [wts[n][:].rearrange("p (j r q) -> p j r q", j=NJ, r=HP)
                   for n in range(BPC)]

            def transform(n, r0, r1):
                xv, wv = xvs[n], wvs[n]

                def dcol(k):
                    return xv[:, r0:r1, k:k + 2 * PAIRS - 1:2]

                nc.vector.tensor_sub(wv[:, 0, r0:r1, :], dcol(0), dcol(2))
                nc.vector.tensor_add(wv[:, 1, r0:r1, :], dcol(1), dcol(2))
                nc.vector.tensor_sub(wv[:, 2, r0:r1, :], dcol(2), dcol(1))
                nc.vector.tensor_sub(wv[:, 3, r0:r1, :], dcol(1), dcol(3))

            def do_strip(n, h, y0, y1):
                wv = wvs[n]
                rows = y1 - y0
                ncols = rows * PAIRS
                ms = []
                for j in range(NJ):
                    pt = ps.tile([128, ncols], f32, tag="acc",
                                 name=f"m_{n}_{h}_{y0}_{j}")
                    for kh in range(KH_EFF):
                        rhs = wv[:, j, y0 + kh:y0 + kh + rows, :]
                        off = ((h * NJ + j) * KH_EFF + kh) * 128
                        nc.tensor.matmul(pt[:], kt[:, off:off + 128], rhs,
                                         start=(kh == 0),
                                         stop=(kh == KH_EFF - 1))
                    ms.append(pt)
                # Evacuate all four points through ACT (PE-W + DVE-R on
                # the same PSUM bank is fatal in HW and ACT-R proved safe in
                # the direct kernel); DVE combines in SBUF only. Bias rides
                # on m1, which reaches both outputs with +1.
                mss = []
                for jj in range(NJ):
                    msj = ev.tile([128, ncols], f32, tag="ev",
                                  name=f"ms_{n}_{h}_{y0}_{jj}")
                    bias_arg = bt[:, h:h + 1] if jj == 1 else 0.0
                    nc.scalar.activation(
                        msj[:], ms[jj][:],
                        mybir.ActivationFunctionType.Identity,
                        bias=bias_arg)
                    mss.append(msj)
                t0 = ev.tile([128, ncols], f32, tag="ev")
                nc.vector.tensor_add(t0[:], mss[0][:], mss[1][:])
                c = ev.tile([128, ncols], f32, tag="ev")
                nc.vector.tensor_sub(c[:], mss[1][:], mss[2][:])
                ot = op.tile([128, rows * W], f32, tag="out")
                ov = ot[:].rearrange("p (r q two) -> p r q two", r=rows, two=2)
                t0v = t0[:].rearrange("p (r q) -> p r q", r=rows)
                m2v = mss[2][:].rearrange("p (r q) -> p r q", r=rows)
                cv = c[:].rearrange("p (r q) -> p r q", r=rows)
                m3v = mss[3][:].rearrange("p (r q) -> p r q", r=rows)
                nc.vector.tensor_add(ov[:, :, :, 0], t0v, m2v)
                nc.vector.tensor_sub(ov[:, :, :, 1], cv, m3v)
                last = (n == BPC - 1 and h == OH - 1 and y1 == H)
                oe = [nc.sync, nc.scalar] if last else [nc.sync, nc.gpsimd]
                for lo, hi in _splits(rows * W, 4 if last else 2):
                    dma(oe, o_d[n, h][:, y0 * W + lo:y0 * W + hi],
                        ot[:, lo:hi])

            # image 0 transforms stream in with the DMAs; image 1's are
            # emitted before its strips
            for r0, r1 in RB:
                transform(0, r0, r1)
            for h in range(OH):
                for y0, y1 in STRIPS:
                    do_strip(0, h, y0, y1)
            for r0, r1 in RB:
                transform(1, r0, r1)
            for h in range(OH):
                for y0, y1 in STRIPS:
                    do_strip(1, h, y0, y1)
    t0 = time.time()
    nc.compile()
    print(f"[kernel] bacc compile: {time.time()-t0:.1f}s", file=sys.stderr)
    return nc


def kernel(x: np.ndarray, weight: np.ndarray, bias: np.ndarray,
           P: np.ndarray) -> np.ndarray:
    global _NC_CACHE, _last_in_maps
    x = np.asarray(x, dtype=np.float32)
    weight = np.asarray(weight, dtype=np.float32)
    bias = np.asarray(bias, dtype=np.float32)
    P = np.asarray(P, dtype=np.float32)

    K = _build_kernel_np(weight, P[0])                    # (O, I, 7, 3)
    if ALGO == "wino":
        # Winograd F(2,3) over kw: 4 points per (o,i,kh);
        # device layout: [i, (oh, j, kh, o128)]
        g = K.reshape(OH, 128, CIN, KH_EFF, K_W)
        gw = np.stack([
            g[..., 0],
            (g[..., 0] + g[..., 1] + g[..., 2]) * 0.5,
            (g[..., 0] - g[..., 1] + g[..., 2]) * 0.5,
            g[..., 2],
        ], axis=1)                                # (OH, 4, 128o, CIN, KH_EFF)
        k_dev = np.ascontiguousarray(
            gw.transpose(3, 0, 1, 4, 2)
            .reshape(CIN, OH * 4 * KH_EFF * 128)).astype(_NP_DT)
    else:
        # device layout: [i, (oh, kh, kw, o128)]
        k_dev = np.ascontiguousarray(
            K.reshape(OH, 128, CIN, KH_EFF, K_W)
            .transpose(2, 0, 3, 4, 1)
            .reshape(CIN, OH * NTAPS * 128)).astype(_NP_DT)

    xpad = np.zeros((B, CIN, HP, WP), np.float32)
    xpad[:, :, PAD_H:PAD_H + H, PAD_W:PAD_W + W] = x
    xpad = xpad.reshape(B, CIN, HP * WP).astype(_NP_DT)

    b_dev = np.ascontiguousarray(bias.reshape(OH, 128, 1))

    if _NC_CACHE is None:
        t0 = time.time()
        _NC_CACHE = (_build_bass_wino() if ALGO == "wino" else _build_bass())
        print(f"[kernel] build+compile total: {time.time()-t0:.1f}s",
              file=sys.stderr)

    in_maps = [
        {"x": np.ascontiguousarray(xpad[i * BPC:(i + 1) * BPC]),
         "k": k_dev, "b": b_dev}
        for i in range(N_CORES)
    ]
    _last_in_maps = in_maps
    t0 = time.time()
    last_exc = None
    for attempt in range(3):
        try:
            res = bass_utils.run_bass_kernel_spmd(
                _NC_CACHE, in_maps, core_ids=list(range(N_CORES)))
            break
        except Exception as e:  # transient device hiccup: retry
            last_exc = e
            print(f"[kernel] run attempt {attempt} failed: {e!r}; retrying",
                  file=sys.stderr)
            time.sleep(5)
    else:
        raise last_exc
    print(f"[kernel] run (incl. walrus compile on first call): "
          f"{time.time()-t0:.1f}s", file=sys.stderr)
    out = np.concatenate(
        [res.results[i]["o"].reshape(BPC, COUT, H, W)
         for i in range(N_CORES)], axis=0)
    return out



# revision 4
# speedup vs baseline: 1.4288x; 1.4288x over previous
"""Dcls2_1d (dilated conv with learnable row spacings) on 8 trn2 NeuronCores.

Strategy: data-parallel over batch (16 -> 2 images/core). Host constructs the
dense (O, I, 7, 3) scattered kernel (exact port of the reference bilinear
scatter) and F(4,3)-transforms it over the width taps; each core runs the conv
as an implicit GEMM contracting over C_in=128 (the partition dim).

Winograd F(4,3) over width: 6 multiply-points per 4 output columns instead of
12, cutting the PE's streamed matmul columns to half of the direct method
(172k cols/core -> ~72us streaming floor at 1 col/cycle/2.4GHz). The 7 height
taps stay direct, accumulated in PSUM per 32-row strip (512-col PSUM banks).

DVE throughput hygiene (TRN2 tensor_tensor only reaches 2x mode for 16-bit
step-1 4B-aligned operands):
 - host pre-splits the padded image into 6 width-phase planes
   (cols 4q+k, k=0..5) so every input-transform read is contiguous fp16;
 - the output transform writes 4 per-phase blocks (contiguous) instead of
   interleaving; the host de-interleaves after the run;
 - m-points are evacuated PSUM->SBUF as fp16 by the scalar engine (bias for
   the whole A^T rides on m1, whose output coefficients are all ones).

Outputs are DMA'd as fp16 (halves store traffic); the host converts to f32.
Measured rel err (max-abs / max|expected|): ~4.5e-3 vs the fp32 reference.

Input DMAs are priority-ordered (first transform block's planes + first tap
weights first), issued from three engine queues in parallel. A burst of
dummy matmuls warms the PE clock gate (HAM) while inputs are in flight.
"""
import os
import sys
import time

sys.path.insert(0, "/opt/trn_rl_repo")

import numpy as np

import concourse.bass as bass
import concourse.tile as tile
from concourse import bacc, mybir
from concourse import bass_utils

# ---- problem constants (hardcoded per contract) ----
K_H, K_W = 3, 3
LIM = 2            # DIL // 2
KH_EFF = 7         # K_H + 2 * LIM
PAD_H, PAD_W = 3, 1
B, CIN, H, W = 16, 128, 64, 64
COUT = 256
N_CORES = 8
BPC = B // N_CORES                  # images per core
HP, WP = H + 2 * PAD_H, W + 2 * PAD_W   # 70, 66
NPIX = H * W                        # 4096
OH = COUT // 128                    # 2 halves of out channels

NJ = 6                              # F(4,3) points
NQ = W // 4                         # output quads per row: 16
NPLANES = 6                         # width-phase input planes
PLANE = HP * NQ                     # cols per plane: 1120
KCOLS = OH * NJ * KH_EFF * 128      # 10752
RB = [(0, 38), (38, 70)]            # input-transform row blocks
STRIPS_STD = [(0, 32), (32, 64)]
STRIPS_LAST = [(0, 32), (32, 48), (48, 64)]   # finer tail on the last block

WARMUP = int(os.environ.get("DCLS_WARMUP", "28"))

MMDT = mybir.dt.float16
NPDT = np.float16

_NC_CACHE = None
_last_in_maps = None  # stashed for test.py's profiled re-run

# F(4,3) weight transform (correlation convention, points 0,+-1,+-2,inf)
G_F43 = np.array([
    [1 / 4, 0, 0],
    [-1 / 6, -1 / 6, -1 / 6],
    [-1 / 6, 1 / 6, -1 / 6],
    [1 / 24, 1 / 12, 1 / 6],
    [1 / 24, -1 / 12, 1 / 6],
    [0, 0, 1],
], np.float32)


def _build_kernel_np(weight: np.ndarray, P1: np.ndarray) -> np.ndarray:
    """Exact numpy port of reference.build_kernel (fp32)."""
    weight = weight.astype(np.float32, copy=False)
    kh = np.arange(K_H, dtype=np.float32)[None, None, :, None]
    pos = kh + LIM + np.clip(P1.astype(np.float32, copy=False), -LIM, LIM)
    p0 = np.floor(pos)
    frac = pos - p0
    p0i = p0.astype(np.int32)
    rng = np.arange(KH_EFF, dtype=np.int32)
    oh0 = (p0i[..., None] == rng).astype(np.float32)
    oh1 = ((p0i + 1)[..., None] == rng).astype(np.float32)
    return (
        np.einsum("oihw,oihwk->oikw", weight * (1.0 - frac), oh0)
        + np.einsum("oihw,oihwk->oikw", weight * frac, oh1)
    ).astype(np.float32)


def _splits(total, n):
    """n near-equal [lo, hi) column ranges covering [0, total)."""
    step = (total + n - 1) // n
    return [(j, min(j + step, total)) for j in range(0, total, step)]


def _strips(n, h):
    return STRIPS_LAST if (n == BPC - 1 and h == OH - 1) else STRIPS_STD


def _build_bass():
    f32 = mybir.dt.float32
    AOP = mybir.AluOpType
    nc = bacc.Bacc("TRN2", target_bir_lowering=False, debug=False,
                   num_devices=N_CORES)
    x_d = nc.dram_tensor("x", [BPC, NPLANES, CIN, PLANE], MMDT,
                         kind="ExternalInput").ap()
    # transformed weights: [i, (oh, j, kh, o128)]
    k_d = nc.dram_tensor("k", [CIN, KCOLS], MMDT, kind="ExternalInput").ap()
    b_d = nc.dram_tensor("b", [OH, 128, 1], f32, kind="ExternalInput").ap()
    # output in per-strip block layout: (strip, phase k, row, quad), fp16
    o_d = nc.dram_tensor("o", [BPC, OH, 128, NPIX], MMDT,
                         kind="ExternalOutput").ap()

    _rr = [0]

    def dma(engines, dst, src):
        eng = engines[_rr[0] % len(engines)]
        _rr[0] += 1
        eng.dma_start(dst, src)

    with tile.TileContext(nc) as tc:
        with tc.tile_pool(name="xp", bufs=1) as xp, \
             tc.tile_pool(name="wp", bufs=1) as wpool, \
             tc.tile_pool(name="kp", bufs=1) as kp, \
             tc.tile_pool(name="bp", bufs=1) as bp, \
             tc.tile_pool(name="wu", bufs=1) as wu, \
             tc.tile_pool(name="tp", bufs=4) as tp, \
             tc.tile_pool(name="ps", bufs=8, space="PSUM") as ps, \
             tc.tile_pool(name="ev", bufs=18) as ev, \
             tc.tile_pool(name="at", bufs=8) as at, \
             tc.tile_pool(name="op", bufs=3) as op:

            kt = kp.tile([CIN, KCOLS], MMDT, tag="k")
            bt = bp.tile([128, OH], f32, tag="bias")
            # 6 phase planes per image: plane p holds cols 4q+p (q=0..15)
            xts = [[xp.tile([CIN, PLANE], MMDT, tag=f"x{n}p{p}",
                            name=f"x{n}p{p}") for p in range(NPLANES)]
                   for n in range(BPC)]
            # transformed planes: [i, (j, row, quad)]
            wts = [wpool.tile([CIN, NJ * PLANE], MMDT, tag=f"w{n}",
                              name=f"w{n}") for n in range(BPC)]

            wt = None
            if WARMUP:
                wt = wu.tile([128, 128], f32, tag="warm")
                nc.vector.memset(wt[:], 0.0)

            # --- input DMAs, priority-ordered, 3 engine queues ---
            ie = [nc.sync, nc.gpsimd, nc.scalar]
            B0C = RB[0][1] * NQ          # cols of transform block 0: 608
            KJ = KH_EFF * 128            # kt cols per (oh, j): 896

            # 1) first weights (oh0, j0) + the planes block0's w0 needs
            dma(ie, kt[:, 0:KJ], k_d[:, 0:KJ])
            for p in (0, 2, 4):
                dma(ie, xts[0][p][:, :B0C], x_d[0, p][:, :B0C])
            # 2) weights (oh0, j1..j2), remaining block0 planes
            dma(ie, kt[:, KJ:3 * KJ], k_d[:, KJ:3 * KJ])
            for p in (1, 3, 5):
                dma(ie, xts[0][p][:, :B0C], x_d[0, p][:, :B0C])
            # 3) img0 block1 planes, weights (oh0, j3..j5)
            for p in range(NPLANES):
                dma(ie, xts[0][p][:, B0C:], x_d[0, p][:, B0C:])
            dma(ie, kt[:, 3 * KJ:6 * KJ], k_d[:, 3 * KJ:6 * KJ])
            # 4) bias, oh1 weights, img1 planes
            for h in range(OH):
                dma(ie, bt[:, h:h + 1], b_d[h])
            for lo, hi in _splits(KCOLS // 2, 3):
                off = KCOLS // 2
                dma(ie, kt[:, off + lo:off + hi], k_d[:, off + lo:off + hi])
            for n in range(1, BPC):
                for p in range(NPLANES):
                    dma(ie, xts[n][p][:], x_d[n, p])

            # --- HAM warmup: dummy matmuls while inputs stream in ---
            for _ in range(WARMUP):
                pw = ps.tile([128, 512], f32, tag="acc")
                nc.tensor.matmul(pw[:, :128], wt[:], wt[:], start=True,
                                 stop=True)

            wvs = [wts[n][:].rearrange("p (j r q) -> p j r q", j=NJ, r=HP)
                   for n in range(BPC)]

            def transform(n, r0, r1):
                """F(4,3) input transform for rows [r0, r1): 6 points from
                planes d0..d5; all reads/writes contiguous fp16 (DVE 2x)."""
                c0, c1 = r0 * NQ, r1 * NQ
                blk = c1 - c0
                d = [xts[n][p][:, c0:c1] for p in range(NPLANES)]
                wv = wvs[n]

                def w(j):
                    return wv[:, j, r0:r1, :]

                cnt = [0]

                def tmp():
                    cnt[0] += 1
                    return tp.tile([CIN, RB[0][1] * NQ], MMDT, tag="t",
                                   name=f"t_{n}_{r0}_{cnt[0]}")

                V = nc.vector
                a = tmp(); V.tensor_sub(a[:, :blk], d[0], d[2])
                b = tmp(); V.tensor_sub(b[:, :blk], d[2], d[4])
                V.scalar_tensor_tensor(w(0), a[:, :blk], 4.0, b[:, :blk],
                                       AOP.mult, AOP.subtract)
                p_ = tmp(); V.tensor_add(p_[:, :blk], d[1], d[2])
                q_ = tmp(); V.tensor_add(q_[:, :blk], d[3], d[4])
                V.scalar_tensor_tensor(w(1), p_[:, :blk], -4.0, q_[:, :blk],
                                       AOP.mult, AOP.add)
                r_ = tmp(); V.tensor_sub(r_[:, :blk], d[1], d[2])
                s_ = tmp(); V.tensor_sub(s_[:, :blk], d[3], d[4])
                V.scalar_tensor_tensor(w(2), r_[:, :blk], 4.0, s_[:, :blk],
                                       AOP.mult, AOP.subtract)
                e_ = tmp(); V.tensor_sub(e_[:, :blk], d[3], d[1])
                f_ = tmp(); V.tensor_sub(f_[:, :blk], d[4], d[2])
                V.scalar_tensor_tensor(w(3), e_[:, :blk], 2.0, f_[:, :blk],
                                       AOP.mult, AOP.add)
                V.scalar_tensor_tensor(w(4), e_[:, :blk], -2.0, f_[:, :blk],
                                       AOP.mult, AOP.add)
                g_ = tmp(); V.tensor_sub(g_[:, :blk], d[3], d[5])
                V.scalar_tensor_tensor(w(5), e_[:, :blk], -4.0, g_[:, :blk],
                                       AOP.mult, AOP.subtract)

            def do_strip(n, h, y0, y1, last):
                rows = y1 - y0
                ncol = rows * NQ
                wv = wvs[n]
                # 6 points, each 7 height taps accumulated in one PSUM bank
                ms = []
                for j in range(NJ):
                    pt = ps.tile([128, 512], mybir.dt.float32, tag="acc",
                                 name=f"m_{n}_{h}_{y0}_{j}")
                    for kh in range(KH_EFF):
                        rhs = wv[:, j, y0 + kh:y0 + kh + rows, :]
                        off = ((h * NJ + j) * KH_EFF + kh) * 128
                        nc.tensor.matmul(pt[:, :ncol], kt[:, off:off + 128],
                                         rhs, start=(kh == 0),
                                         stop=(kh == KH_EFF - 1))
                    mj = ev.tile([128, 512], MMDT, tag="ev",
                                 name=f"ms_{n}_{h}_{y0}_{j}")
                    if j == 1:
                        # A^T's m1 column is all-ones: bias rides here
                        nc.scalar.activation(
                            mj[:, :ncol], pt[:, :ncol],
                            mybir.ActivationFunctionType.Identity,
                            bias=bt[:, h:h + 1])
                    else:
                        nc.scalar.activation(
                            mj[:, :ncol], pt[:, :ncol],
                            mybir.ActivationFunctionType.Copy)
                    ms.append(mj[:, :ncol])

                # output transform on DVE (all fp16 contiguous, 2x mode)
                # y0=m0+m1+m2+m3+m4; y1=m1-m2+2(m3-m4);
                # y2=m1+m2+4(m3+m4); y3=m1-m2+8(m3-m4)+m5
                cnt = [0]

                def tmp():
                    cnt[0] += 1
                    return at.tile([128, 512], MMDT, tag="a",
                                   name=f"a_{n}_{h}_{y0}_{cnt[0]}")

                V = nc.vector
                ot = op.tile([128, 4 * 512], MMDT, tag="out",
                             name=f"ot_{n}_{h}_{y0}")
                ov = ot[:, :4 * ncol].rearrange("p (k c) -> p k c", k=4)
                s1 = tmp(); V.tensor_add(s1[:, :ncol], ms[1], ms[2])
                d1 = tmp(); V.tensor_sub(d1[:, :ncol], ms[1], ms[2])
                t1 = tmp(); V.tensor_add(t1[:, :ncol], ms[3], ms[4])
                u1 = tmp(); V.tensor_sub(u1[:, :ncol], ms[3], ms[4])
                v_ = tmp(); V.tensor_add(v_[:, :ncol], ms[0], s1[:, :ncol])
                V.tensor_add(ov[:, 0, :], v_[:, :ncol], t1[:, :ncol])
                V.scalar_tensor_tensor(ov[:, 1, :], u1[:, :ncol], 2.0,
                                       d1[:, :ncol], AOP.mult, AOP.add)
                V.scalar_tensor_tensor(ov[:, 2, :], t1[:, :ncol], 4.0,
                                       s1[:, :ncol], AOP.mult, AOP.add)
                w_ = tmp(); V.tensor_add(w_[:, :ncol], d1[:, :ncol], ms[5])
                V.scalar_tensor_tensor(ov[:, 3, :], u1[:, :ncol], 8.0,
                                       w_[:, :ncol], AOP.mult, AOP.add)

                # store: (strip rows) * 64 output cols, block layout
                base = y0 * W
                tot = 4 * ncol
                if last:
                    oe = [nc.sync, nc.scalar]
                    for lo, hi in _splits(tot, 2):
                        dma(oe, o_d[n, h][:, base + lo:base + hi],
                            ot[:, lo:hi])
                else:
                    dma([nc.sync, nc.gpsimd], o_d[n, h][:, base:base + tot],
                        ot[:, :tot])

            # transforms for both images lead the DVE queue (they only
            # depend on input DMAs); assemblies follow with their deps
            for n in range(BPC):
                for r0, r1 in RB:
                    transform(n, r0, r1)
            for n in range(BPC):
                for h in range(OH):
                    ss = _strips(n, h)
                    for si, (y0, y1) in enumerate(ss):
                        is_last = (n == BPC - 1 and h == OH - 1
                                   and si == len(ss) - 1)
                        do_strip(n, h, y0, y1, is_last)
    t0 = time.time()
    nc.compile()
    print(f"[kernel] bacc compile: {time.time()-t0:.1f}s", file=sys.stderr)
    return nc


def _host_prep(x, weight, bias, P):
    K = _build_kernel_np(weight, P[0])                    # (O, I, 7, 3)
    g = K.reshape(OH, 128, CIN, KH_EFF, K_W)
    gw = np.einsum('jw,moikw->mjoik', G_F43, g)           # (OH,NJ,o,i,kh)
    k_dev = np.ascontiguousarray(
        gw.transpose(3, 0, 1, 4, 2).reshape(CIN, KCOLS)).astype(NPDT)

    xpad = np.zeros((B, CIN, HP, WP), np.float32)
    xpad[:, :, PAD_H:PAD_H + H, PAD_W:PAD_W + W] = x
    xpad = xpad.astype(NPDT)
    xpl = np.empty((B, NPLANES, CIN, HP, NQ), NPDT)
    for k in range(NPLANES):
        xpl[:, k] = xpad[:, :, :, k::4][:, :, :, :NQ]
    xpl = xpl.reshape(B, NPLANES, CIN, PLANE)

    b_dev = np.ascontiguousarray(bias.reshape(OH, 128, 1)).astype(np.float32)
    return xpl, k_dev, b_dev


def _unpack_core(o_core):
    """(BPC, OH, 128, 4096) fp16 block layout -> (BPC, 256, 64, 64) f32."""
    out = np.empty((BPC, OH, 128, H, W), np.float32)
    for n in range(BPC):
        for h in range(OH):
            for (y0, y1) in _strips(n, h):
                rows = y1 - y0
                blk = o_core[n, h][:, y0 * W:y1 * W].reshape(
                    128, 4, rows, NQ).astype(np.float32)
                # [c, k, r, q] -> [c, r, q, k] -> (c, rows, 64)
                out[n, h, :, y0:y1, :] = blk.transpose(0, 2, 3, 1).reshape(
                    128, rows, W)
    return out.reshape(BPC, COUT, H, W)


def kernel(x: np.ndarray, weight: np.ndarray, bias: np.ndarray,
           P: np.ndarray) -> np.ndarray:
    global _NC_CACHE, _last_in_maps
    x = np.asarray(x, dtype=np.float32)
    weight = np.asarray(weight, dtype=np.float32)
    bias = np.asarray(bias, dtype=np.float32)
    P = np.asarray(P, dtype=np.float32)

    xpl, k_dev, b_dev = _host_prep(x, weight, bias, P)

    if _NC_CACHE is None:
        t0 = time.time()
        _NC_CACHE = _build_bass()
        print(f"[kernel] build+compile total: {time.time()-t0:.1f}s",
              file=sys.stderr)

    in_maps = [
        {"x": np.ascontiguousarray(xpl[i * BPC:(i + 1) * BPC]),
         "k": k_dev, "b": b_dev}
        for i in range(N_CORES)
    ]
    _last_in_maps = in_maps
    t0 = time.time()
    last_exc = None
    for attempt in range(3):
        try:
            res = bass_utils.run_bass_kernel_spmd(
                _NC_CACHE, in_maps, core_ids=list(range(N_CORES)))
            break
        except Exception as e:  # transient device hiccup: retry
            last_exc = e
            print(f"[kernel] run attempt {attempt} failed: {e!r}; retrying",
                  file=sys.stderr)
            time.sleep(5)
    else:
        raise last_exc
    print(f"[kernel] run (incl. walrus compile on first call): "
          f"{time.time()-t0:.1f}s", file=sys.stderr)
    out = np.concatenate(
        [_unpack_core(res.results[i]["o"].reshape(BPC, OH, 128, NPIX))
         for i in range(N_CORES)], axis=0)
    return out
